# revision 9
# baseline (speedup 1.0000x reference)
"""Single-head attention with QKV projections for TRN2, batch-sharded across
8 NeuronCores (one batch element per core).

Reference computation per batch element (S=2048, D=1024, fp32):
    Q = xq @ Wq + bq ; K = xk @ Wk + bk ; V = xv @ Wv + bv
    L = Q @ K^T                      # [S, S]
    out = (softmax(L, -1) * 1/sqrt(D)) @ V

Per-core plan (all matmuls fp32r = full-rate fp32 on the PE; empirically
~310ns per 128x128x512 matmul incl. the serial weight load):
  Phase A-Q: xq tiles -> PE-transpose (f32r) -> xq^T ; Q^T = Wq^T @ xq^T
             -> DRAM scratch [D, S]
  Phase A-K: K^T = Wk^T @ xk^T -> resident SBUF [D, S]
  Phase A-V: V = xv @ Wv       -> resident SBUF [S, D]
  Phase B (per 512-col q strip of Q^T):
    L^T tiles [sk=128, sq=512] = K^T_tile.T @ Q^T-strip   (PSUM)
    U^T = exp(L^T) on ACT straight out of PSUM (no max subtraction: |L| < ~80
      so exp stays inside fp32 range), written as f32r
    rowsum[1, sq] += ones[128,1].T @ U^T_t   (PE, accumulated over sk tiles)
    rowsumT[sq-tile, 1] = rowsum_slice.T via K=1 matmul; recipT = (1/32)/rowsumT
    out[sq, d] = U^T_slice.T @ V  accumulated over sk tiles, normalized by
      per-partition recipT via DVE tensor_scalar, + bv broadcast.
"""
import numpy as np
from contextlib import ExitStack

import concourse.bass as bass
import concourse.bacc as bacc
import concourse.tile as tile
import concourse.mybir as mybir
from concourse.bass_utils import run_bass_kernel_spmd

F32 = mybir.dt.float32
F32R = mybir.dt.float32r
AF = mybir.ActivationFunctionType

B, S, D = 8, 2048, 1024
NKT = D // 128          # 8 contraction tiles
NST = S // 128          # 16 s tiles
SCALE = 1.0 / 32.0      # 1/sqrt(D)

_CACHED = {}


def build(nrep=1, barrier=False):
    nc = bacc.Bacc("TRN2", target_bir_lowering=False, debug=False, num_devices=8)

    xq = nc.dram_tensor("xq", [S, D], F32R, kind="ExternalInput")
    xk = nc.dram_tensor("xk", [S, D], F32R, kind="ExternalInput")
    xv = nc.dram_tensor("xv", [S, D], F32R, kind="ExternalInput")
    wq = nc.dram_tensor("wq", [D, D], F32R, kind="ExternalInput")
    wk = nc.dram_tensor("wk", [D, D], F32R, kind="ExternalInput")
    wv = nc.dram_tensor("wv", [D, D], F32R, kind="ExternalInput")
    bqd = nc.dram_tensor("bqd", [128, NKT], F32, kind="ExternalInput")  # bq.reshape(8,128).T
    bkd = nc.dram_tensor("bkd", [128, NKT], F32, kind="ExternalInput")
    bvd = nc.dram_tensor("bvd", [1, D], F32R, kind="ExternalInput")
    identd = nc.dram_tensor("identd", [128, 128], F32R, kind="ExternalInput")
    ones1d = nc.dram_tensor("ones1d", [1, 128], F32R, kind="ExternalInput")
    onespd = nc.dram_tensor("onespd", [128, 1], F32R, kind="ExternalInput")

    out = nc.dram_tensor("out", [S, D], F32, kind="ExternalOutput")
    qt_dram = nc.dram_tensor("qt_scratch", [D, S], F32R)  # internal scratch
    ut_dram = nc.dram_tensor("ut_scratch", [S, S], F32R)  # exp(L^T) spill

    with tile.TileContext(nc) as tc, ExitStack() as ctx:
        # ---------------- persistent pools ----------------
        cpool = ctx.enter_context(tc.tile_pool(name="const", bufs=1))
        pp = ctx.enter_context(tc.tile_pool(name="pp", bufs=3, space="PSUM"))
        op = ctx.enter_context(tc.tile_pool(name="op", bufs=3, space="PSUM"))

        ident = cpool.tile([128, 128], F32R, tag="ident")
        bqs = cpool.tile([128, NKT], F32, tag="bqs")
        bks = cpool.tile([128, NKT], F32, tag="bks")
        bvs = cpool.tile([1, D], F32R, tag="bvs")
        ones1 = cpool.tile([1, 128], F32R, tag="ones1")
        onesp = cpool.tile([128, 1], F32R, tag="onesp")
        bvb = cpool.tile([128, D], F32, tag="bvb")
        nc.gpsimd.dma_start(ident[:], identd.ap())
        nc.gpsimd.dma_start(bqs[:], bqd.ap())
        nc.gpsimd.dma_start(bks[:], bkd.ap())
        nc.gpsimd.dma_start(bvs[:], bvd.ap())
        nc.gpsimd.dma_start(ones1[:], ones1d.ap())
        nc.gpsimd.dma_start(onespd_s := onesp[:], onespd.ap())

        # broadcast bv across partitions via K=1 matmul: bvb = ones1.T @ bvs
        for h in range(2):
            bps = op.tile([128, 512], F32, tag="av")
            nc.tensor.matmul(bps[:], ones1[:], bvs[:, h * 512:(h + 1) * 512],
                             start=True, stop=True)
            nc.scalar.copy(bvb[:, h * 512:(h + 1) * 512], bps[:])

        # ---------------- phase A: projections ----------------
        rep_stack = ExitStack()
        def load_w(wpool, w_dram):
            w_s = wpool.tile([128, NKT * D], F32R, tag="w")
            for k in range(NKT):
                nc.gpsimd.dma_start(w_s[:, k * D:(k + 1) * D],
                                  w_dram.ap()[k * 128:(k + 1) * 128, :])
            return w_s

        def transpose_strip(tp, xpool, xtpool, x_dram, j, n_stiles):
            """Load x rows [j*128*n .. ) and produce x^T strip [D, 128*n] (f32r)."""
            xt = xtpool.tile([128, NKT * 128 * n_stiles], F32R, tag="xt")
            for st in range(n_stiles):
                xl = xpool.tile([128, D], F32R, tag="xl")
                nc.sync.dma_start(
                    xl[:], x_dram.ap()[(j * n_stiles + st) * 128:(j * n_stiles + st + 1) * 128, :])
                for k4 in range(NKT // 4):
                    tpt = tp.tile([128, 512], F32R, tag="tp")
                    for kk in range(4):
                        k = k4 * 4 + kk
                        nc.tensor.transpose(tpt[:, kk * 128:(kk + 1) * 128],
                                            xl[:, k * 128:(k + 1) * 128], ident[:])
                    # scatter 4 transposed tiles into xt at (k, st) slots
                    dst = xt[:].rearrange("p (k s) -> p k s", s=128 * n_stiles)
                    nc.vector.tensor_copy(
                        dst[:, k4 * 4:k4 * 4 + 4, st * 128:(st + 1) * 128], tpt[:])
            return xt

        for _rep in range(nrep):
          if _rep and barrier:
              tc.strict_bb_all_engine_barrier()
          kctx = ExitStack()
          ktp = kctx.enter_context(tc.tile_pool(name="ktr", bufs=1))
          with ExitStack() as actx:
            tp = actx.enter_context(tc.tile_pool(name="tp", bufs=2, space="PSUM"))
            wpool = actx.enter_context(tc.tile_pool(name="wpool", bufs=1))
            xpool = actx.enter_context(tc.tile_pool(name="xpool", bufs=2))
            xtpool = actx.enter_context(tc.tile_pool(name="xtpool", bufs=1))

            # ---- A-Q: Q^T -> DRAM scratch ----
            with nc.named_scope("phase_aq"), ExitStack() as qctx:
                qstg = qctx.enter_context(tc.tile_pool(name="qstg", bufs=3))
                w_s = load_w(wpool, wq)
                for j in range(4):
                    xt = transpose_strip(tp, xpool, xtpool, xq, j, 4)
                    for m in range(NKT):
                        ppt = pp.tile([128, 512], F32, tag="pp")
                        for k in range(NKT):
                            nc.tensor.matmul(
                                ppt[:],
                                w_s[:, k * D + m * 128:k * D + (m + 1) * 128],
                                xt[:, k * 512:(k + 1) * 512],
                                start=(k == 0), stop=(k == NKT - 1))
                        qs_t = qstg.tile([128, 512], F32R, tag="qs")
                        nc.scalar.activation(qs_t[:], ppt[:], AF.Identity,
                                             bias=bqs[:, m:m + 1])
                        nc.scalar.dma_start(
                            qt_dram.ap()[m * 128:(m + 1) * 128, j * 512:(j + 1) * 512],
                            qs_t[:])

            # ---- A-K: K^T resident ----
            kt = ktp.tile([128, NKT * S], F32R, tag="kt")       # K^T resident
            with nc.named_scope("phase_ak"):
                w_s = load_w(wpool, wk)
                for j in range(4):
                    xt = transpose_strip(tp, xpool, xtpool, xk, j, 4)
                    for m in range(NKT):
                        ppt = pp.tile([128, 512], F32, tag="pp")
                        for k in range(NKT):
                            nc.tensor.matmul(
                                ppt[:],
                                w_s[:, k * D + m * 128:k * D + (m + 1) * 128],
                                xt[:, k * 512:(k + 1) * 512],
                                start=(k == 0), stop=(k == NKT - 1))
                        nc.scalar.activation(
                            kt[:, m * S + j * 512:m * S + (j + 1) * 512],
                            ppt[:], AF.Identity, bias=bks[:, m:m + 1])


        # ---------------- phase B: attention (transposed logits) ----------------
          # ---- B1: logits + exp -> U^T spill, rowsums ----
          rssums = cpool.tile([1, S], F32R, tag="rssums")
          with ExitStack() as bctx, nc.named_scope("phase_b1"):
            qsp = bctx.enter_context(tc.tile_pool(name="qsp", bufs=1))
            utsg = bctx.enter_context(tc.tile_pool(name="utsg", bufs=3))
            rsps = bctx.enter_context(tc.tile_pool(name="rsps", bufs=1, space="PSUM"))

            for j in range(4):                  # q strips of 512
                qs = qsp.tile([128, NKT * 512], F32R, tag="qs")
                src_ap = qt_dram.ap()[:, j * 512:(j + 1) * 512]
                nc.sync.dma_start(
                    qs[:].rearrange("p (k s) -> p k s", s=512),
                    src_ap.rearrange("(k p) s -> p k s", p=128))

                rs_ps = rsps.tile([1, 512], F32, tag="rs")
                for t in range(NST):
                    lpt = pp.tile([128, 512], F32, tag="pp")
                    for k in range(NKT):
                        nc.tensor.matmul(
                            lpt[:],
                            kt[:, k * S + t * 128:k * S + (t + 1) * 128],
                            qs[:, k * 512:(k + 1) * 512],
                            start=(k == 0), stop=(k == NKT - 1))
                    ut_t = utsg.tile([128, 512], F32R, tag="uts")
                    nc.scalar.activation(ut_t[:], lpt[:], AF.Exp)
                    nc.scalar.dma_start(
                        ut_dram.ap()[t * 128:(t + 1) * 128, j * 512:(j + 1) * 512],
                        ut_t[:])
                    nc.tensor.matmul(rs_ps[:], onesp[:], ut_t[:],
                                     start=(t == 0), stop=(t == NST - 1))
                nc.scalar.copy(rssums[:, j * 512:(j + 1) * 512], rs_ps[:])

          kctx.close()

          # ---- B2: out = (U^T.T @ xv) @ Wv, normalized ----
          with ExitStack() as bctx, nc.named_scope("phase_b2"):
            xvp = bctx.enter_context(tc.tile_pool(name="xvp", bufs=1))
            wp2 = bctx.enter_context(tc.tile_pool(name="wp2", bufs=1))
            utp = bctx.enter_context(tc.tile_pool(name="utp", bufs=1))
            o2p = bctx.enter_context(tc.tile_pool(name="o2p", bufs=1))
            osp = bctx.enter_context(tc.tile_pool(name="osp", bufs=2))
            rsp = bctx.enter_context(tc.tile_pool(name="rsp", bufs=2))
            rtps = bctx.enter_context(tc.tile_pool(name="rtps", bufs=1, space="PSUM"))

            xv_res = xvp.tile([128, NST * D], F32R, tag="xvres")
            for t in range(NST):
                nc.sync.dma_start(xv_res[:, t * D:(t + 1) * D],
                                  xv.ap()[t * 128:(t + 1) * 128, :])
            wv_s = wp2.tile([128, NKT * D], F32R, tag="wv2")
            for k in range(NKT):
                nc.gpsimd.dma_start(wv_s[:, k * D:(k + 1) * D],
                                    wv.ap()[k * 128:(k + 1) * 128, :])

            for j in range(4):                  # q strips of 512
                uts = utp.tile([128, NST * 512], F32R, tag="uts2")
                nc.sync.dma_start(
                    uts[:].rearrange("p (t s) -> p t s", s=512),
                    ut_dram.ap()[:, j * 512:(j + 1) * 512]
                        .rearrange("(t p) s -> p t s", p=128))

                # out2^T strip [D, 512]: contraction over sk
                o2t = o2p.tile([128, NKT * 512], F32R, tag="o2t")
                for dt in range(NKT):
                    ppt = pp.tile([128, 512], F32, tag="pp")
                    for t in range(NST):
                        nc.tensor.matmul(
                            ppt[:],
                            xv_res[:, t * D + dt * 128:t * D + (dt + 1) * 128],
                            uts[:, t * 512:(t + 1) * 512],
                            start=(t == 0), stop=(t == NST - 1))
                    nc.vector.tensor_copy(o2t[:, dt * 512:(dt + 1) * 512], ppt[:])

                for m in range(4):              # q tiles of 128 within strip
                    sq = j * 4 + m
                    rt_ps = rtps.tile([128, 2], F32, tag="rt")
                    nc.tensor.matmul(rt_ps[:],
                                     rssums[:, sq * 128:(sq + 1) * 128],
                                     ones1[:, 0:2], start=True, stop=True)
                    rct = rsp.tile([128, 1], F32, tag="rct")
                    nc.vector.reciprocal(rct[:], rt_ps[:, 0:1])
                    nc.vector.tensor_scalar_mul(rct[:], rct[:], SCALE)

                    os_t = osp.tile([128, D], F32, tag="os")
                    for h in range(2):
                        opt = op.tile([128, 512], F32, tag="av")
                        for k in range(NKT):
                            nc.tensor.matmul(
                                opt[:],
                                o2t[:, k * 512 + m * 128:k * 512 + (m + 1) * 128],
                                wv_s[:, k * D + h * 512:k * D + (h + 1) * 512],
                                start=(k == 0), stop=(k == NKT - 1))
                        nc.vector.tensor_scalar_mul(
                            os_t[:, h * 512:(h + 1) * 512], opt[:], rct[:])
                    nc.vector.tensor_add(os_t[:], os_t[:], bvb[:])
                    nc.scalar.dma_start(out.ap()[sq * 128:(sq + 1) * 128, :], os_t[:])
    nc.compile()
    return nc


def _get_nc():
    if "nc" not in _CACHED:
        _CACHED["nc"] = build()
    return _CACHED["nc"]


def make_in_maps(q, k, v, Wq, bq, Wk, bk, Wv, bv):
    q = np.ascontiguousarray(q, np.float32)
    k = np.ascontiguousarray(k, np.float32)
    v = np.ascontiguousarray(v, np.float32)
    consts = {
        "wq": np.ascontiguousarray(Wq, np.float32),
        "wk": np.ascontiguousarray(Wk, np.float32),
        "wv": np.ascontiguousarray(Wv, np.float32),
        "bqd": np.ascontiguousarray(np.asarray(bq, np.float32).reshape(NKT, 128).T),
        "bkd": np.ascontiguousarray(np.asarray(bk, np.float32).reshape(NKT, 128).T),
        "bvd": np.asarray(bv, np.float32).reshape(1, D).copy(),
        "identd": np.eye(128, dtype=np.float32),
        "ones1d": np.ones((1, 128), np.float32),
        "onespd": np.ones((128, 1), np.float32),
    }
    return [dict(consts, xq=q[c], xk=k[c], xv=v[c]) for c in range(B)]


def kernel(q, k, v, Wq, bq, Wk, bk, Wv, bv, _trace=False, _trace_kwargs=None):
    in_maps = make_in_maps(q, k, v, Wq, bq, Wk, bk, Wv, bv)
    nc = _get_nc()
    res = run_bass_kernel_spmd(nc, in_maps, core_ids=list(range(B)),
                               trace=_trace, **(_trace_kwargs or {}))
    out = np.stack([res.results[c]["out"] for c in range(B)])
    if _trace:
        kernel.last_results = res
    return out


# revision 10
# speedup vs baseline: 1.2319x; 1.2319x over previous
"""Single-head attention with QKV projections for TRN2, batch-sharded across
8 NeuronCores (one batch element per core).

Reference computation per batch element (S=2048, D=1024, fp32):
    Q = xq @ Wq + bq ; K = xk @ Wk + bk ; V = xv @ Wv + bv
    L = Q @ K^T                      # [S, S]
    out = (softmax(L, -1) * 1/sqrt(D)) @ V

Per-core plan (all matmuls fp32r = full-rate fp32 on the PE; empirically
~310ns per 128x128x512 matmul incl. the serial weight load):
  Phase A-Q: xq tiles -> PE-transpose (f32r) -> xq^T ; Q^T = Wq^T @ xq^T
             -> DRAM scratch [D, S]
  Phase A-K: K^T = Wk^T @ xk^T -> resident SBUF [D, S]
  Phase A-V: V = xv @ Wv       -> resident SBUF [S, D]
  Phase B (per 512-col q strip of Q^T):
    L^T tiles [sk=128, sq=512] = K^T_tile.T @ Q^T-strip   (PSUM)
    U^T = exp(L^T) on ACT straight out of PSUM (no max subtraction: |L| < ~80
      so exp stays inside fp32 range), written as f32r
    rowsum[1, sq] += ones[128,1].T @ U^T_t   (PE, accumulated over sk tiles)
    rowsumT[sq-tile, 1] = rowsum_slice.T via K=1 matmul; recipT = (1/32)/rowsumT
    out[sq, d] = U^T_slice.T @ V  accumulated over sk tiles, normalized by
      per-partition recipT via DVE tensor_scalar, + bv broadcast.
"""
import numpy as np
from contextlib import ExitStack

import concourse.bass as bass
import concourse.bacc as bacc
import concourse.tile as tile
import concourse.mybir as mybir
from concourse.bass_utils import run_bass_kernel_spmd

F32 = mybir.dt.float32
F32R = mybir.dt.float32r
AF = mybir.ActivationFunctionType

B, S, D = 8, 2048, 1024
NKT = D // 128          # 8 contraction tiles
NST = S // 128          # 16 s tiles
SCALE = 1.0 / 32.0      # 1/sqrt(D)

_CACHED = {}


def build(nrep=1, barrier=False):
    nc = bacc.Bacc("TRN2", target_bir_lowering=False, debug=False, num_devices=8)

    xq = nc.dram_tensor("xq", [S, D], F32R, kind="ExternalInput")
    xk = nc.dram_tensor("xk", [S, D], F32R, kind="ExternalInput")
    xv = nc.dram_tensor("xv", [S, D], F32R, kind="ExternalInput")
    wq = nc.dram_tensor("wq", [D, D], F32R, kind="ExternalInput")
    wk = nc.dram_tensor("wk", [D, D], F32R, kind="ExternalInput")
    wv = nc.dram_tensor("wv", [D, D], F32R, kind="ExternalInput")
    bqd = nc.dram_tensor("bqd", [128, NKT], F32, kind="ExternalInput")  # bq.reshape(8,128).T
    bkd = nc.dram_tensor("bkd", [128, NKT], F32, kind="ExternalInput")
    bvd = nc.dram_tensor("bvd", [1, D], F32R, kind="ExternalInput")
    identd = nc.dram_tensor("identd", [128, 128], F32R, kind="ExternalInput")
    ones1d = nc.dram_tensor("ones1d", [1, 128], F32R, kind="ExternalInput")
    onespd = nc.dram_tensor("onespd", [128, 1], F32R, kind="ExternalInput")

    out = nc.dram_tensor("out", [S, D], F32, kind="ExternalOutput")
    qt_dram = nc.dram_tensor("qt_scratch", [D, S], F32R)  # internal scratch

    with tile.TileContext(nc) as tc, ExitStack() as ctx:
        # ---------------- persistent pools ----------------
        cpool = ctx.enter_context(tc.tile_pool(name="const", bufs=1))
        ktp = ctx.enter_context(tc.tile_pool(name="ktr", bufs=1))
        vsp = ctx.enter_context(tc.tile_pool(name="vres", bufs=1))
        pp = ctx.enter_context(tc.tile_pool(name="pp", bufs=3, space="PSUM"))
        op = ctx.enter_context(tc.tile_pool(name="op", bufs=3, space="PSUM"))

        ident = cpool.tile([128, 128], F32R, tag="ident")
        bqs = cpool.tile([128, NKT], F32, tag="bqs")
        bks = cpool.tile([128, NKT], F32, tag="bks")
        bvs = cpool.tile([1, D], F32R, tag="bvs")
        ones1 = cpool.tile([1, 128], F32R, tag="ones1")
        onesp = cpool.tile([128, 1], F32R, tag="onesp")
        bvb = cpool.tile([128, D], F32, tag="bvb")
        nc.gpsimd.dma_start(ident[:], identd.ap())
        nc.gpsimd.dma_start(bqs[:], bqd.ap())
        nc.gpsimd.dma_start(bks[:], bkd.ap())
        nc.gpsimd.dma_start(bvs[:], bvd.ap())
        nc.gpsimd.dma_start(ones1[:], ones1d.ap())
        nc.gpsimd.dma_start(onespd_s := onesp[:], onespd.ap())

        # broadcast bv across partitions via K=1 matmul: bvb = ones1.T @ bvs
        for h in range(2):
            bps = op.tile([128, 512], F32, tag="av")
            nc.tensor.matmul(bps[:], ones1[:], bvs[:, h * 512:(h + 1) * 512],
                             start=True, stop=True)
            nc.scalar.copy(bvb[:, h * 512:(h + 1) * 512], bps[:])

        # ---------------- phase A: projections ----------------
        rep_stack = ExitStack()
        def load_w(wpool, w_dram):
            w_s = wpool.tile([128, NKT * D], F32R, tag="w")
            for k in range(NKT):
                nc.gpsimd.dma_start(w_s[:, k * D:(k + 1) * D],
                                  w_dram.ap()[k * 128:(k + 1) * 128, :])
            return w_s

        def transpose_strip(tp, xpool, xtpool, x_dram, j, n_stiles):
            """Load x rows [j*128*n .. ) and produce x^T strip [D, 128*n] (f32r)."""
            xt = xtpool.tile([128, NKT * 128 * n_stiles], F32R, tag="xt")
            for st in range(n_stiles):
                xl = xpool.tile([128, D], F32R, tag="xl")
                nc.sync.dma_start(
                    xl[:], x_dram.ap()[(j * n_stiles + st) * 128:(j * n_stiles + st + 1) * 128, :])
                for k4 in range(NKT // 4):
                    tpt = tp.tile([128, 512], F32R, tag="tp")
                    for kk in range(4):
                        k = k4 * 4 + kk
                        nc.tensor.transpose(tpt[:, kk * 128:(kk + 1) * 128],
                                            xl[:, k * 128:(k + 1) * 128], ident[:])
                    # scatter 4 transposed tiles into xt at (k, st) slots
                    dst = xt[:].rearrange("p (k s) -> p k s", s=128 * n_stiles)
                    nc.vector.tensor_copy(
                        dst[:, k4 * 4:k4 * 4 + 4, st * 128:(st + 1) * 128], tpt[:])
            return xt

        for _rep in range(nrep):
          if _rep and barrier:
              tc.strict_bb_all_engine_barrier()
          with ExitStack() as actx:
            tp = actx.enter_context(tc.tile_pool(name="tp", bufs=2, space="PSUM"))
            wpool = actx.enter_context(tc.tile_pool(name="wpool", bufs=1))
            xpool = actx.enter_context(tc.tile_pool(name="xpool", bufs=2))
            xtpool = actx.enter_context(tc.tile_pool(name="xtpool", bufs=1))

            # ---- A-Q: Q^T -> DRAM scratch ----
            with nc.named_scope("phase_aq"), ExitStack() as qctx:
                qstg = qctx.enter_context(tc.tile_pool(name="qstg", bufs=3))
                w_s = load_w(wpool, wq)
                for j in range(4):
                    xt = transpose_strip(tp, xpool, xtpool, xq, j, 4)
                    for m in range(NKT):
                        ppt = pp.tile([128, 512], F32, tag="pp")
                        for k in range(NKT):
                            nc.tensor.matmul(
                                ppt[:],
                                w_s[:, k * D + m * 128:k * D + (m + 1) * 128],
                                xt[:, k * 512:(k + 1) * 512],
                                start=(k == 0), stop=(k == NKT - 1))
                        qs_t = qstg.tile([128, 512], F32R, tag="qs")
                        nc.scalar.activation(qs_t[:], ppt[:], AF.Identity,
                                             bias=bqs[:, m:m + 1])
                        nc.scalar.dma_start(
                            qt_dram.ap()[m * 128:(m + 1) * 128, j * 512:(j + 1) * 512],
                            qs_t[:])

            # ---- A-K: K^T resident ----
            kt = ktp.tile([128, NKT * S], F32R, tag="kt")       # K^T resident
            with nc.named_scope("phase_ak"):
                w_s = load_w(wpool, wk)
                for j in range(4):
                    xt = transpose_strip(tp, xpool, xtpool, xk, j, 4)
                    for m in range(NKT):
                        ppt = pp.tile([128, 512], F32, tag="pp")
                        for k in range(NKT):
                            nc.tensor.matmul(
                                ppt[:],
                                w_s[:, k * D + m * 128:k * D + (m + 1) * 128],
                                xt[:, k * 512:(k + 1) * 512],
                                start=(k == 0), stop=(k == NKT - 1))
                        nc.scalar.activation(
                            kt[:, m * S + j * 512:m * S + (j + 1) * 512],
                            ppt[:], AF.Identity, bias=bks[:, m:m + 1])

            # ---- A-V: V resident ----
            vs = vsp.tile([128, NST * D], F32R, tag="vs")       # V resident
            with nc.named_scope("phase_av"):
                w_s = load_w(wpool, wv)
                for j in range(4):
                    xt = transpose_strip(tp, xpool, xtpool, xv, j, 4)
                    for m in range(4):          # s tiles within strip
                        sg = j * 4 + m
                        for h in range(2):      # dout halves
                            ppt = pp.tile([128, 512], F32, tag="pp")
                            for k in range(NKT):
                                nc.tensor.matmul(
                                    ppt[:],
                                    xt[:, k * 512 + m * 128:k * 512 + (m + 1) * 128],
                                    w_s[:, k * D + h * 512:k * D + (h + 1) * 512],
                                    start=(k == 0), stop=(k == NKT - 1))
                            nc.vector.tensor_copy(
                                vs[:, sg * D + h * 512:sg * D + (h + 1) * 512], ppt[:])

        # ---------------- phase B: attention (transposed logits) ----------------
          with ExitStack() as bctx, nc.named_scope("phase_b"):
            qsp = bctx.enter_context(tc.tile_pool(name="qsp", bufs=1))
            utp = bctx.enter_context(tc.tile_pool(name="utp", bufs=1))
            osp = bctx.enter_context(tc.tile_pool(name="osp", bufs=2))
            rsp = bctx.enter_context(tc.tile_pool(name="rsp", bufs=2))
            rsps = bctx.enter_context(tc.tile_pool(name="rsps", bufs=1, space="PSUM"))
            rtps = bctx.enter_context(tc.tile_pool(name="rtps", bufs=1, space="PSUM"))

            for j in range(4):                  # q strips of 512
                qs = qsp.tile([128, NKT * 512], F32R, tag="qs")
                src = qt_dram.ap()[:, j * 512:(j + 1) * 512]
                nc.sync.dma_start(
                    qs[:].rearrange("p (k s) -> p k s", s=512),
                    src.rearrange("(k p) s -> p k s", p=128))

                # L^T tiles + exp -> U^T strip [S, 512] (f32r)
                ut = utp.tile([128, NST * 512], F32R, tag="ut")
                for t in range(NST):
                    lpt = pp.tile([128, 512], F32, tag="pp")
                    for k in range(NKT):
                        nc.tensor.matmul(
                            lpt[:],
                            kt[:, k * S + t * 128:k * S + (t + 1) * 128],
                            qs[:, k * 512:(k + 1) * 512],
                            start=(k == 0), stop=(k == NKT - 1))
                    nc.scalar.activation(ut[:, t * 512:(t + 1) * 512],
                                         lpt[:], AF.Exp)

                # rowsum over sk (partition dim) via ones matmuls -> [1, 512]
                rs_ps = rsps.tile([1, 512], F32, tag="rs")
                for t in range(NST):
                    nc.tensor.matmul(rs_ps[:], onesp[:],
                                     ut[:, t * 512:(t + 1) * 512],
                                     start=(t == 0), stop=(t == NST - 1))
                rs_sb = rsp.tile([1, 512], F32R, tag="rssb")
                nc.scalar.copy(rs_sb[:], rs_ps[:])

                for m in range(4):              # q tiles of 128 within strip
                    sq = j * 4 + m
                    # rowsumT [128,1] via K=1 matmul, then recipT = (1/32)/rowsum
                    rt_ps = rtps.tile([128, 2], F32, tag="rt")
                    nc.tensor.matmul(rt_ps[:],
                                     rs_sb[:, m * 128:(m + 1) * 128],
                                     ones1[:, 0:2], start=True, stop=True)
                    rct = rsp.tile([128, 1], F32, tag="rct")
                    nc.vector.reciprocal(rct[:], rt_ps[:, 0:1])
                    nc.vector.tensor_scalar_mul(rct[:], rct[:], SCALE)

                    # out[sq, :] = U^T_slice.T @ V, normalized + bv
                    os_t = osp.tile([128, D], F32, tag="os")
                    for h in range(2):
                        opt = op.tile([128, 512], F32, tag="av")
                        for t in range(NST):
                            nc.tensor.matmul(
                                opt[:],
                                ut[:, t * 512 + m * 128:t * 512 + (m + 1) * 128],
                                vs[:, t * D + h * 512:t * D + (h + 1) * 512],
                                start=(t == 0), stop=(t == NST - 1))
                        nc.vector.tensor_scalar_mul(
                            os_t[:, h * 512:(h + 1) * 512], opt[:], rct[:])
                    nc.vector.tensor_add(os_t[:], os_t[:], bvb[:])
                    nc.scalar.dma_start(out.ap()[sq * 128:(sq + 1) * 128, :], os_t[:])

    nc.compile()
    return nc


def _get_nc():
    if "nc" not in _CACHED:
        _CACHED["nc"] = build()
    return _CACHED["nc"]


def make_in_maps(q, k, v, Wq, bq, Wk, bk, Wv, bv):
    q = np.ascontiguousarray(q, np.float32)
    k = np.ascontiguousarray(k, np.float32)
    v = np.ascontiguousarray(v, np.float32)
    consts = {
        "wq": np.ascontiguousarray(Wq, np.float32),
        "wk": np.ascontiguousarray(Wk, np.float32),
        "wv": np.ascontiguousarray(Wv, np.float32),
        "bqd": np.ascontiguousarray(np.asarray(bq, np.float32).reshape(NKT, 128).T),
        "bkd": np.ascontiguousarray(np.asarray(bk, np.float32).reshape(NKT, 128).T),
        "bvd": np.asarray(bv, np.float32).reshape(1, D).copy(),
        "identd": np.eye(128, dtype=np.float32),
        "ones1d": np.ones((1, 128), np.float32),
        "onespd": np.ones((128, 1), np.float32),
    }
    return [dict(consts, xq=q[c], xk=k[c], xv=v[c]) for c in range(B)]


def kernel(q, k, v, Wq, bq, Wk, bk, Wv, bv, _trace=False, _trace_kwargs=None):
    in_maps = make_in_maps(q, k, v, Wq, bq, Wk, bk, Wv, bv)
    nc = _get_nc()
    res = run_bass_kernel_spmd(nc, in_maps, core_ids=list(range(B)),
                               trace=_trace, **(_trace_kwargs or {}))
    out = np.stack([res.results[c]["out"] for c in range(B)])
    if _trace:
        kernel.last_results = res
    return out
